# revision 12
# baseline (speedup 1.0000x reference)
"""Trainium2 Bass kernel for nn_Attention (non-local-block style attention).

Reference computation (per batch b, z flattened to [Ci, N], N = T*H*W = 4096):
    theta = w_theta @ z + b_theta        [Co, N]
    phi   = w_phi   @ z + b_phi          [Co, N]
    psi   = w_psi   @ z + b_psi          [Co, N]
    g[n,m]   = sum_c phi[c,n] psi[c,m]
    G        = relu(g / N)
    tmp[c,n] = sum_m G[n,m] theta[c,m]
    out      = w_v @ tmp + b_v + z       [Ci, N]

Sharding: 8 cores = 2 batches x 4 token-blocks of N/4=1024. Each core gets the
full z[b] (needed for psi/theta over all m) plus its own token block, computes
out[b][:, blk]. Fully data-parallel SPMD, no collectives.

Per-core dataflow. The attention matmuls run in bf16 (full PE rate, FWL weight
loads) accumulating into fp32 PSUM; the 1/N normalization is folded into w_psi
host-side; the residual path stays fp32.
  psi_dup [128, 4096] bf16: duplicated-weight projection (rows 0-63 == 64-127)
  phi_dup [128, 1024] bf16: same for phi on the token block
  thT     [128, 32*64] bf16: theta^T tiles (m on partitions), theta bias added
                        via ones-row matmul prefill of each PSUM bank
  loop over 32 m-tiles:
    gT[m_tile] [128, 1024] = psi[:, m_slice]^T . phi   (K=64 matmuls)
    G = relu(gT) PSUM->SBUF bf16 on ScalarE/VectorE (alternating)
    tmp[64, 1024] += thT[m_tile]^T . G                 (K=128, PSUM-accumulated)
  vg = w_v^T . tmp (f32r) ; out = vg + b_v + z_blk (fp32) ; DMA out.
"""

import ml_dtypes
import numpy as np

import concourse.bacc as bacc
import concourse.mybir as mybir
import concourse.tile as tile
from concourse.bass_utils import run_bass_kernel_spmd

F32 = mybir.dt.float32
F32R = mybir.dt.float32r
BF16 = mybir.dt.bfloat16
AF = mybir.ActivationFunctionType
ALU = mybir.AluOpType
BF16NP = ml_dtypes.bfloat16

B, CI, CO = 2, 128, 64
T, H, W = 4, 32, 32
N = T * H * W            # 4096 tokens
NCORES = 8
BLK = N // (NCORES // B)  # 1024 tokens per core
CH = 512                 # psum-bank chunk
MT = N // 128            # 32 m-tiles

_CACHE = {}


def _build():
    nc = bacc.Bacc("TRN2", target_bir_lowering=False, debug=False)

    zb16 = nc.dram_tensor("zb16", [CI, N], BF16, kind="ExternalInput")
    zblk16 = nc.dram_tensor("zblk16", [CI, BLK], BF16, kind="ExternalInput")
    zblk = nc.dram_tensor("zblk", [CI, BLK], F32, kind="ExternalInput")
    wpsiT2 = nc.dram_tensor("wpsiT2", [CI, 128], BF16, kind="ExternalInput")
    wphiT2 = nc.dram_tensor("wphiT2", [CI, 128], BF16, kind="ExternalInput")
    wthetaT = nc.dram_tensor("wthetaT", [CI, CO], BF16, kind="ExternalInput")
    wvT2 = nc.dram_tensor("wvT2", [128, CI], F32R, kind="ExternalInput")
    bpsi2 = nc.dram_tensor("bpsi2", [128, 1], F32, kind="ExternalInput")
    bphi2 = nc.dram_tensor("bphi2", [128, 1], F32, kind="ExternalInput")
    btheta8 = nc.dram_tensor("btheta8", [1, CH], BF16, kind="ExternalInput")
    onesr = nc.dram_tensor("onesr", [1, CI], BF16, kind="ExternalInput")
    bv = nc.dram_tensor("bv", [CI, 1], F32, kind="ExternalInput")
    out = nc.dram_tensor("out", [CI, BLK], F32, kind="ExternalOutput")

    with tile.TileContext(nc) as tc:
        with (
            tc.tile_pool(name="const", bufs=1) as cpool,
            tc.tile_pool(name="zp", bufs=1) as zp,
            tc.tile_pool(name="proj", bufs=1) as pp,
            tc.tile_pool(name="gs", bufs=8) as gp,
            tc.tile_pool(name="tail", bufs=2) as tailp,
            tc.tile_pool(name="pst", bufs=1, space="PSUM") as pst,
        ):
            # ---- loads: first what the warm-up burst and psi need ----
            wpsiT2_sb = cpool.tile([CI, 128], BF16)
            nc.sync.dma_start(wpsiT2_sb[:], wpsiT2[:])
            zb16_sb = zp.tile([CI, N], BF16)
            for j in range(N // CH):
                nc.sync.dma_start(
                    zb16_sb[:, j * CH:(j + 1) * CH], zb16[:, j * CH:(j + 1) * CH]
                )
            wphiT2_sb = cpool.tile([CI, 128], BF16)
            nc.gpsimd.dma_start(wphiT2_sb[:], wphiT2[:])
            wthetaT_sb = cpool.tile([CI, CO], BF16)
            nc.gpsimd.dma_start(wthetaT_sb[:], wthetaT[:])
            btheta8_sb = cpool.tile([1, CH], BF16)
            nc.gpsimd.dma_start(btheta8_sb[:], btheta8[:])
            ones_sb = cpool.tile([1, CI], BF16)
            nc.gpsimd.dma_start(ones_sb[:], onesr[:])
            bpsi_sb = cpool.tile([128, 1], F32)
            nc.gpsimd.dma_start(bpsi_sb[:], bpsi2[:])
            bphi_sb = cpool.tile([128, 1], F32)
            nc.gpsimd.dma_start(bphi_sb[:], bphi2[:])
            zblk16_sb = zp.tile([CI, BLK], BF16)
            nc.sync.dma_start(zblk16_sb[:], zblk16[:])
            wvT2_sb = cpool.tile([128, CI], F32R)
            nc.gpsimd.dma_start(wvT2_sb[:], wvT2[:])
            bv_sb = cpool.tile([CI, 1], F32)
            nc.gpsimd.dma_start(bv_sb[:], bv[:])
            zblk_sb = zp.tile([CI, BLK], F32)
            for j in range(BLK // CH):
                nc.sync.dma_start(
                    zblk_sb[:, j * CH:(j + 1) * CH], zblk[:, j * CH:(j + 1) * CH]
                )

            # tmp accumulator: one PSUM bank, col-packed
            # rows 0:64 = tmp[:, 0:512], rows 64:128 = tmp[:, 512:1024]
            tmp_ps = pst.tile([128, CH], F32)

            psi_sb = pp.tile([128, N], BF16)
            phi_sb = pp.tile([128, BLK], BF16)
            thT_sb = pp.tile([128, MT * CO], BF16)

            # ---- HAM ignition: dense same-weight matmuls during the DMA
            # phase push the PE activity monitor over its busy threshold so
            # the array clock ramps 1.2 -> 2.4 GHz before the real work.
            with tc.tile_pool(name="warm", bufs=1, space="PSUM") as wpool:
                wps = wpool.tile([128, CH], F32)
                for k in range(10):
                    nc.tensor.matmul(
                        wps[:], wpsiT2_sb[:], zb16_sb[:, 0:CH],
                        skip_group_check=True,
                    )

            with tc.tile_pool(name="psm", bufs=3, space="PSUM") as psm:
                # ---- psi_dup [128, N] bf16 (pre-scaled by 1/N host-side) ----
                for j in range(N // CH):
                    ps = psm.tile([128, CH], F32, tag="m", name=f"psi{j}")
                    nc.tensor.matmul(
                        ps[:], wpsiT2_sb[:], zb16_sb[:, j * CH:(j + 1) * CH]
                    )
                    dst = psi_sb[:, j * CH:(j + 1) * CH]
                    if j % 2 == 0:
                        nc.scalar.activation(dst, ps[:], AF.Identity, bias=bpsi_sb[:])
                    else:
                        nc.vector.tensor_scalar_add(dst, ps[:], bpsi_sb[:])

                # ---- phi_dup [128, BLK] bf16 ----
                for j in range(BLK // CH):
                    ps = psm.tile([128, CH], F32, tag="m", name=f"phi{j}")
                    nc.tensor.matmul(
                        ps[:], wphiT2_sb[:], zblk16_sb[:, j * CH:(j + 1) * CH]
                    )
                    dst = phi_sb[:, j * CH:(j + 1) * CH]
                    if j % 2 == 0:
                        nc.scalar.activation(dst, ps[:], AF.Identity, bias=bphi_sb[:])
                    else:
                        nc.vector.tensor_scalar_add(dst, ps[:], bphi_sb[:])

                # ---- thT [128, MT*64] bf16: theta^T tiles with bias prefill ----
                for grp in range(4):
                    ps = psm.tile([128, CH], F32, tag="m", name=f"th{grp}")
                    nc.tensor.matmul(
                        ps[:], ones_sb[:], btheta8_sb[:],
                        start=True, stop=False, skip_group_check=True,
                    )
                    for j in range(8):
                        mi = grp * 8 + j
                        nc.tensor.matmul(
                            ps[:, j * CO:(j + 1) * CO],
                            zb16_sb[:, mi * 128:(mi + 1) * 128],
                            wthetaT_sb[:],
                            start=False, stop=(j == 7), skip_group_check=True,
                        )
                    dst = thT_sb[:, grp * CH:(grp + 1) * CH]
                    if grp % 2 == 0:
                        nc.vector.tensor_copy(dst, ps[:])
                    else:
                        nc.scalar.activation(dst, ps[:], AF.Copy)

            # ---- main attention loop: 16 pair-iterations over 32 m-tiles ----
            # g matmuls row-packed (K=64 pairs -> full array + HAM-warm);
            # tmp matmuls col-packed (M=64 pairs). One PSUM bank per g chunk
            # with a 7-deep rotation keeps the PE gap-free.
            with tc.tile_pool(name="psg", bufs=7, space="PSUM") as psg:
                for it in range(MT // 2):
                    mA, mB = 2 * it, 2 * it + 1
                    gps = {}
                    for h in range(2):
                        hs = slice(h * CH, (h + 1) * CH)
                        gps[0, h] = psg.tile(
                            [128, CH], F32, tag="g", name=f"gA{it}_{h}"
                        )
                        nc.tensor.matmul(
                            gps[0, h][:],
                            psi_sb[0:CO, mA * 128:(mA + 1) * 128],
                            phi_sb[0:CO, hs],
                            tile_position=(0, 0),
                        )
                        gps[1, h] = psg.tile(
                            [128, CH], F32, tag="g", name=f"gB{it}_{h}"
                        )
                        nc.tensor.matmul(
                            gps[1, h][:],
                            psi_sb[CO:128, mB * 128:(mB + 1) * 128],
                            phi_sb[CO:128, hs],
                            tile_position=(64, 0),
                        )
                    gsb = {}
                    for ab in range(2):
                        for h in range(2):
                            t = gp.tile(
                                [128, CH], BF16, tag="gs", name=f"s{ab}_{it}_{h}"
                            )
                            gsb[ab, h] = t
                            on_act = ab == 0 or (h == 0 and it % 4 == 3)
                            if on_act:
                                nc.scalar.activation(t[:], gps[ab, h][:], AF.Relu)
                            else:
                                nc.vector.tensor_scalar_max(
                                    t[:], gps[ab, h][:], 0.0
                                )
                    for ab, mt in ((0, mA), (1, mB)):
                        lhs = thT_sb[:, mt * CO:(mt + 1) * CO]
                        nc.tensor.matmul(
                            tmp_ps[0:CO, :], lhs, gsb[ab, 0][:],
                            start=(mt == 0), stop=(mt == MT - 1),
                            tile_position=(0, 0), skip_group_check=True,
                        )
                        nc.tensor.matmul(
                            tmp_ps[CO:128, :], lhs, gsb[ab, 1][:],
                            start=(mt == 0), stop=(mt == MT - 1),
                            tile_position=(0, 64), skip_group_check=True,
                        )

            # ---- tail: tmp -> SBUF, vg = w_v^T tmp (row-packed), out ----
            with tc.tile_pool(name="psv", bufs=2, space="PSUM") as psv:
                tmp_sb = tailp.tile([128, CH], F32R, tag="tmp")
                nc.scalar.activation(tmp_sb[:], tmp_ps[:], AF.Copy)
                vgA = psv.tile([CI, CH], F32, tag="v", name="vgA")
                vgB = psv.tile([CI, CH], F32, tag="v", name="vgB")
                nc.tensor.matmul(
                    vgA[:], wvT2_sb[0:CO, :], tmp_sb[0:CO, :], tile_position=(0, 0)
                )
                nc.tensor.matmul(
                    vgB[:], wvT2_sb[CO:128, :], tmp_sb[CO:128, :],
                    tile_position=(64, 0),
                )
                for h, vg_ps in ((0, vgA), (1, vgB)):
                    out_sb = tailp.tile([CI, CH], F32, tag="os", name=f"os{h}")
                    nc.vector.scalar_tensor_tensor(
                        out_sb[:],
                        vg_ps[:],
                        bv_sb[:],
                        zblk_sb[:, h * CH:(h + 1) * CH],
                        ALU.add,
                        ALU.add,
                    )
                    nc.sync.dma_start(out[:, h * CH:(h + 1) * CH], out_sb[:])

    nc.compile()
    return nc


def _get_nc():
    if "nc" not in _CACHE:
        _CACHE["nc"] = _build()
    return _CACHE["nc"]


def build_in_maps(z, w_theta, b_theta, w_phi, b_phi, w_psi, b_psi, w_v, b_v):
    z = np.asarray(z, dtype=np.float32)
    z2 = np.ascontiguousarray(z.reshape(B, CI, N))
    z216 = z2.astype(BF16NP)

    sc = np.float32(1.0 / N)
    wpsiT = np.asarray(w_psi, np.float32).T * sc
    wphiT = np.asarray(w_phi, np.float32).T
    wpsiT2 = np.ascontiguousarray(
        np.concatenate([wpsiT, wpsiT], axis=1).astype(BF16NP)
    )
    wphiT2 = np.ascontiguousarray(
        np.concatenate([wphiT, wphiT], axis=1).astype(BF16NP)
    )
    wthetaT = np.ascontiguousarray(np.asarray(w_theta, np.float32).T.astype(BF16NP))
    wvT1 = np.asarray(w_v, np.float32).T
    wvT2 = np.ascontiguousarray(np.concatenate([wvT1, wvT1], axis=0))
    bpsi2 = np.concatenate([b_psi, b_psi]).astype(np.float32)[:, None] * sc
    bphi2 = np.concatenate([b_phi, b_phi]).astype(np.float32)[:, None]
    btheta8 = np.ascontiguousarray(
        np.tile(np.asarray(b_theta, np.float32), 8)[None, :].astype(BF16NP)
    )
    bvc = np.ascontiguousarray(np.asarray(b_v, np.float32)[:, None])
    ones = np.ones((1, CI), dtype=BF16NP)

    in_maps = []
    for core in range(NCORES):
        b, nb = divmod(core, NCORES // B)
        sl = slice(nb * BLK, (nb + 1) * BLK)
        in_maps.append(
            {
                "zb16": z216[b],
                "zblk16": np.ascontiguousarray(z216[b][:, sl]),
                "zblk": np.ascontiguousarray(z2[b][:, sl]),
                "wpsiT2": wpsiT2,
                "wphiT2": wphiT2,
                "wthetaT": wthetaT,
                "wvT2": wvT2,
                "bpsi2": bpsi2,
                "bphi2": bphi2,
                "btheta8": btheta8,
                "onesr": ones,
                "bv": bvc,
            }
        )
    return in_maps


def kernel(z, w_theta, b_theta, w_phi, b_phi, w_psi, b_psi, w_v, b_v):
    in_maps = build_in_maps(
        z, w_theta, b_theta, w_phi, b_phi, w_psi, b_psi, w_v, b_v
    )
    nc = _get_nc()
    res = run_bass_kernel_spmd(nc, in_maps, core_ids=list(range(NCORES)))

    out_full = np.empty((B, CI, N), dtype=np.float32)
    for core in range(NCORES):
        b, nb = divmod(core, NCORES // B)
        out_full[b][:, nb * BLK:(nb + 1) * BLK] = res.results[core]["out"]
    return out_full.reshape(B, CI, T, H, W)


# revision 13
# speedup vs baseline: 1.1377x; 1.1377x over previous
"""Trainium2 Bass kernel for nn_Attention (non-local-block style attention).

Reference computation (per batch b, z flattened to [Ci, N], N = T*H*W = 4096):
    theta = w_theta @ z + b_theta        [Co, N]
    phi   = w_phi   @ z + b_phi          [Co, N]
    psi   = w_psi   @ z + b_psi          [Co, N]
    g[n,m]   = sum_c phi[c,n] psi[c,m]
    G        = relu(g / N)
    tmp[c,n] = sum_m G[n,m] theta[c,m]
    out      = w_v @ tmp + b_v + z       [Ci, N]

Sharding: 8 cores = 2 batches x 4 token-blocks of N/4=1024. Each core gets the
full z[b] (needed for psi/theta over all m) plus its own token block, computes
out[b][:, blk]. Fully data-parallel SPMD, no collectives.

Per-core dataflow. The attention matmuls run in bf16 (full PE rate, FWL weight
loads) accumulating into fp32 PSUM; the 1/N normalization is folded into w_psi
host-side; the residual path stays fp32.
  psi_dup [128, 4096] bf16: duplicated-weight projection (rows 0-63 == 64-127)
  phi_dup [128, 1024] bf16: same for phi on the token block
  thT     [128, 32*64] bf16: theta^T tiles (m on partitions), theta bias added
                        via ones-row matmul prefill of each PSUM bank
  loop over 32 m-tiles:
    gT[m_tile] [128, 1024] = psi[:, m_slice]^T . phi   (K=64 matmuls)
    G = relu(gT) PSUM->SBUF bf16 on ScalarE/VectorE (alternating)
    tmp[64, 1024] += thT[m_tile]^T . G                 (K=128, PSUM-accumulated)
  vg = w_v^T . tmp (f32r) ; out = vg + b_v + z_blk (fp32) ; DMA out.
"""

import ml_dtypes
import numpy as np

import concourse.bacc as bacc
import concourse.mybir as mybir
import concourse.tile as tile
from concourse.bass_utils import run_bass_kernel_spmd

F32 = mybir.dt.float32
F32R = mybir.dt.float32r
BF16 = mybir.dt.bfloat16
AF = mybir.ActivationFunctionType
ALU = mybir.AluOpType
BF16NP = ml_dtypes.bfloat16

B, CI, CO = 2, 128, 64
T, H, W = 4, 32, 32
N = T * H * W            # 4096 tokens
NCORES = 8
BLK = N // (NCORES // B)  # 1024 tokens per core
CH = 512                 # psum-bank chunk
MT = N // 128            # 32 m-tiles

_CACHE = {}


def _build():
    nc = bacc.Bacc("TRN2", target_bir_lowering=False, debug=False)

    zb16 = nc.dram_tensor("zb16", [CI, N], BF16, kind="ExternalInput")
    zblk16 = nc.dram_tensor("zblk16", [CI, BLK], BF16, kind="ExternalInput")
    zblk = nc.dram_tensor("zblk", [CI, BLK], F32, kind="ExternalInput")
    wpsiT2 = nc.dram_tensor("wpsiT2", [CI, 128], BF16, kind="ExternalInput")
    wphiT2 = nc.dram_tensor("wphiT2", [CI, 128], BF16, kind="ExternalInput")
    wthetaT = nc.dram_tensor("wthetaT", [CI, CO], BF16, kind="ExternalInput")
    wvT2 = nc.dram_tensor("wvT2", [128, CI], F32R, kind="ExternalInput")
    bpsi2 = nc.dram_tensor("bpsi2", [128, 1], F32, kind="ExternalInput")
    bphi2 = nc.dram_tensor("bphi2", [128, 1], F32, kind="ExternalInput")
    btheta8 = nc.dram_tensor("btheta8", [1, CH], BF16, kind="ExternalInput")
    onesr = nc.dram_tensor("onesr", [1, CI], BF16, kind="ExternalInput")
    bv = nc.dram_tensor("bv", [CI, 1], F32, kind="ExternalInput")
    out = nc.dram_tensor("out", [CI, BLK], F32, kind="ExternalOutput")

    with tile.TileContext(nc) as tc:
        with (
            tc.tile_pool(name="const", bufs=1) as cpool,
            tc.tile_pool(name="zp", bufs=1) as zp,
            tc.tile_pool(name="proj", bufs=1) as pp,
            tc.tile_pool(name="gs", bufs=4) as gp,
            tc.tile_pool(name="tail", bufs=2) as tailp,
            tc.tile_pool(name="pst", bufs=1, space="PSUM") as pst,
        ):
            # ---- loads: first what the warm-up burst and psi need ----
            wpsiT2_sb = cpool.tile([CI, 128], BF16)
            nc.sync.dma_start(wpsiT2_sb[:], wpsiT2[:])
            zb16_sb = zp.tile([CI, N], BF16)
            for j in range(N // CH):
                nc.sync.dma_start(
                    zb16_sb[:, j * CH:(j + 1) * CH], zb16[:, j * CH:(j + 1) * CH]
                )
            wphiT2_sb = cpool.tile([CI, 128], BF16)
            nc.gpsimd.dma_start(wphiT2_sb[:], wphiT2[:])
            wthetaT_sb = cpool.tile([CI, CO], BF16)
            nc.gpsimd.dma_start(wthetaT_sb[:], wthetaT[:])
            btheta8_sb = cpool.tile([1, CH], BF16)
            nc.gpsimd.dma_start(btheta8_sb[:], btheta8[:])
            ones_sb = cpool.tile([1, CI], BF16)
            nc.gpsimd.dma_start(ones_sb[:], onesr[:])
            bpsi_sb = cpool.tile([128, 1], F32)
            nc.gpsimd.dma_start(bpsi_sb[:], bpsi2[:])
            bphi_sb = cpool.tile([128, 1], F32)
            nc.gpsimd.dma_start(bphi_sb[:], bphi2[:])
            zblk16_sb = zp.tile([CI, BLK], BF16)
            nc.sync.dma_start(zblk16_sb[:], zblk16[:])
            wvT2_sb = cpool.tile([128, CI], F32R)
            nc.gpsimd.dma_start(wvT2_sb[:], wvT2[:])
            bv_sb = cpool.tile([CI, 1], F32)
            nc.gpsimd.dma_start(bv_sb[:], bv[:])
            zblk_sb = zp.tile([CI, BLK], F32)
            for j in range(BLK // CH):
                nc.sync.dma_start(
                    zblk_sb[:, j * CH:(j + 1) * CH], zblk[:, j * CH:(j + 1) * CH]
                )

            # tmp accumulator: one PSUM bank, col-packed
            # rows 0:64 = tmp[:, 0:512], rows 64:128 = tmp[:, 512:1024]
            tmp_ps = pst.tile([128, CH], F32)

            psi_sb = pp.tile([128, N], BF16)
            phi_sb = pp.tile([128, BLK], BF16)
            thT_sb = pp.tile([128, MT * CO], BF16)

            # ---- HAM ignition: dense same-weight matmuls during the DMA
            # phase push the PE activity monitor over its busy threshold so
            # the array clock ramps 1.2 -> 2.4 GHz before the real work.
            with tc.tile_pool(name="warm", bufs=1, space="PSUM") as wpool:
                wps = wpool.tile([128, CH], F32)
                for k in range(10):
                    nc.tensor.matmul(
                        wps[:], wpsiT2_sb[:], zb16_sb[:, 0:CH],
                        skip_group_check=True,
                    )

            with tc.tile_pool(name="psm", bufs=3, space="PSUM") as psm:
                # ---- psi_dup [128, N] bf16 (pre-scaled by 1/N host-side) ----
                for j in range(N // CH):
                    ps = psm.tile([128, CH], F32, tag="m", name=f"psi{j}")
                    nc.tensor.matmul(
                        ps[:], wpsiT2_sb[:], zb16_sb[:, j * CH:(j + 1) * CH]
                    )
                    dst = psi_sb[:, j * CH:(j + 1) * CH]
                    if j % 2 == 0:
                        nc.scalar.activation(dst, ps[:], AF.Identity, bias=bpsi_sb[:])
                    else:
                        nc.vector.tensor_scalar_add(dst, ps[:], bpsi_sb[:])

                # ---- phi_dup [128, BLK] bf16 ----
                for j in range(BLK // CH):
                    ps = psm.tile([128, CH], F32, tag="m", name=f"phi{j}")
                    nc.tensor.matmul(
                        ps[:], wphiT2_sb[:], zblk16_sb[:, j * CH:(j + 1) * CH]
                    )
                    dst = phi_sb[:, j * CH:(j + 1) * CH]
                    if j % 2 == 0:
                        nc.scalar.activation(dst, ps[:], AF.Identity, bias=bphi_sb[:])
                    else:
                        nc.vector.tensor_scalar_add(dst, ps[:], bphi_sb[:])

                # ---- thT [128, MT*64] bf16: theta^T tiles with bias prefill ----
                for grp in range(4):
                    ps = psm.tile([128, CH], F32, tag="m", name=f"th{grp}")
                    nc.tensor.matmul(
                        ps[:], ones_sb[:], btheta8_sb[:],
                        start=True, stop=False, skip_group_check=True,
                    )
                    for j in range(8):
                        mi = grp * 8 + j
                        nc.tensor.matmul(
                            ps[:, j * CO:(j + 1) * CO],
                            zb16_sb[:, mi * 128:(mi + 1) * 128],
                            wthetaT_sb[:],
                            start=False, stop=(j == 7), skip_group_check=True,
                        )
                    dst = thT_sb[:, grp * CH:(grp + 1) * CH]
                    if grp % 2 == 0:
                        nc.vector.tensor_copy(dst, ps[:])
                    else:
                        nc.scalar.activation(dst, ps[:], AF.Copy)

            # ---- main attention loop: 16 pair-iterations over 32 m-tiles ----
            # g matmuls row-packed (K=64 pairs -> full array + HAM-warm);
            # tmp matmuls col-packed (M=64 pairs).
            with tc.tile_pool(name="psg", bufs=3, space="PSUM") as psg:
                for it in range(MT // 2):
                    mA, mB = 2 * it, 2 * it + 1
                    gpsA = psg.tile([128, 2 * CH], F32, tag="g", name=f"gA{it}")
                    gpsB = psg.tile([128, 2 * CH], F32, tag="g", name=f"gB{it}")
                    for h in range(2):
                        hs = slice(h * CH, (h + 1) * CH)
                        nc.tensor.matmul(
                            gpsA[:, hs],
                            psi_sb[0:CO, mA * 128:(mA + 1) * 128],
                            phi_sb[0:CO, hs],
                            tile_position=(0, 0),
                        )
                        nc.tensor.matmul(
                            gpsB[:, hs],
                            psi_sb[CO:128, mB * 128:(mB + 1) * 128],
                            phi_sb[CO:128, hs],
                            tile_position=(64, 0),
                        )
                    gsbA = gp.tile([128, 2 * CH], BF16, tag="gs", name=f"sA{it}")
                    gsbB = gp.tile([128, 2 * CH], BF16, tag="gs", name=f"sB{it}")
                    nc.scalar.activation(gsbA[:], gpsA[:], AF.Relu)
                    if it in (5, 11):
                        nc.scalar.activation(gsbB[:], gpsB[:], AF.Relu)
                    else:
                        nc.vector.tensor_scalar_max(gsbB[:], gpsB[:], 0.0)
                    for mt, gsb in ((mA, gsbA), (mB, gsbB)):
                        lhs = thT_sb[:, mt * CO:(mt + 1) * CO]
                        nc.tensor.matmul(
                            tmp_ps[0:CO, :], lhs, gsb[:, 0:CH],
                            start=(mt == 0), stop=(mt == MT - 1),
                            tile_position=(0, 0), skip_group_check=True,
                        )
                        nc.tensor.matmul(
                            tmp_ps[CO:128, :], lhs, gsb[:, CH:2 * CH],
                            start=(mt == 0), stop=(mt == MT - 1),
                            tile_position=(0, 64), skip_group_check=True,
                        )

            # ---- tail: tmp -> SBUF, vg = w_v^T tmp (row-packed), out ----
            with tc.tile_pool(name="psv", bufs=2, space="PSUM") as psv:
                tmp_sb = tailp.tile([128, CH], F32R, tag="tmp")
                nc.scalar.activation(tmp_sb[:], tmp_ps[:], AF.Copy)
                vgA = psv.tile([CI, CH], F32, tag="v", name="vgA")
                vgB = psv.tile([CI, CH], F32, tag="v", name="vgB")
                nc.tensor.matmul(
                    vgA[:], wvT2_sb[0:CO, :], tmp_sb[0:CO, :], tile_position=(0, 0)
                )
                nc.tensor.matmul(
                    vgB[:], wvT2_sb[CO:128, :], tmp_sb[CO:128, :],
                    tile_position=(64, 0),
                )
                for h, vg_ps in ((0, vgA), (1, vgB)):
                    out_sb = tailp.tile([CI, CH], F32, tag="os", name=f"os{h}")
                    nc.vector.scalar_tensor_tensor(
                        out_sb[:],
                        vg_ps[:],
                        bv_sb[:],
                        zblk_sb[:, h * CH:(h + 1) * CH],
                        ALU.add,
                        ALU.add,
                    )
                    nc.sync.dma_start(out[:, h * CH:(h + 1) * CH], out_sb[:])

    nc.compile()
    return nc


def _get_nc():
    if "nc" not in _CACHE:
        _CACHE["nc"] = _build()
    return _CACHE["nc"]


def build_in_maps(z, w_theta, b_theta, w_phi, b_phi, w_psi, b_psi, w_v, b_v):
    z = np.asarray(z, dtype=np.float32)
    z2 = np.ascontiguousarray(z.reshape(B, CI, N))
    z216 = z2.astype(BF16NP)

    sc = np.float32(1.0 / N)
    wpsiT = np.asarray(w_psi, np.float32).T * sc
    wphiT = np.asarray(w_phi, np.float32).T
    wpsiT2 = np.ascontiguousarray(
        np.concatenate([wpsiT, wpsiT], axis=1).astype(BF16NP)
    )
    wphiT2 = np.ascontiguousarray(
        np.concatenate([wphiT, wphiT], axis=1).astype(BF16NP)
    )
    wthetaT = np.ascontiguousarray(np.asarray(w_theta, np.float32).T.astype(BF16NP))
    wvT1 = np.asarray(w_v, np.float32).T
    wvT2 = np.ascontiguousarray(np.concatenate([wvT1, wvT1], axis=0))
    bpsi2 = np.concatenate([b_psi, b_psi]).astype(np.float32)[:, None] * sc
    bphi2 = np.concatenate([b_phi, b_phi]).astype(np.float32)[:, None]
    btheta8 = np.ascontiguousarray(
        np.tile(np.asarray(b_theta, np.float32), 8)[None, :].astype(BF16NP)
    )
    bvc = np.ascontiguousarray(np.asarray(b_v, np.float32)[:, None])
    ones = np.ones((1, CI), dtype=BF16NP)

    in_maps = []
    for core in range(NCORES):
        b, nb = divmod(core, NCORES // B)
        sl = slice(nb * BLK, (nb + 1) * BLK)
        in_maps.append(
            {
                "zb16": z216[b],
                "zblk16": np.ascontiguousarray(z216[b][:, sl]),
                "zblk": np.ascontiguousarray(z2[b][:, sl]),
                "wpsiT2": wpsiT2,
                "wphiT2": wphiT2,
                "wthetaT": wthetaT,
                "wvT2": wvT2,
                "bpsi2": bpsi2,
                "bphi2": bphi2,
                "btheta8": btheta8,
                "onesr": ones,
                "bv": bvc,
            }
        )
    return in_maps


def kernel(z, w_theta, b_theta, w_phi, b_phi, w_psi, b_psi, w_v, b_v):
    in_maps = build_in_maps(
        z, w_theta, b_theta, w_phi, b_phi, w_psi, b_psi, w_v, b_v
    )
    nc = _get_nc()
    res = run_bass_kernel_spmd(nc, in_maps, core_ids=list(range(NCORES)))

    out_full = np.empty((B, CI, N), dtype=np.float32)
    for core in range(NCORES):
        b, nb = divmod(core, NCORES // B)
        out_full[b][:, nb * BLK:(nb + 1) * BLK] = res.results[core]["out"]
    return out_full.reshape(B, CI, T, H, W)
